# revision 25
# baseline (speedup 1.0000x reference)
"""Trainium2 Bass kernel for nn_AttentionLayer (per-pixel attention + 3x3 conv).

Problem (per batch b):
    query = W1 @ img + b1                       # [Ck=64, HW]
    scores[hw, l] = sum_k query[k, hw] v[k, l]  # [HW, L=256]
    att = softmax(scores, axis=l)
    value[c, hw] = sum_l att[hw, l] v[c, l]     # [64, HW]
    cat = [img; value]                          # [320, HW]
    out = conv3x3(cat, W2) + b2                 # [256, H, W], padding=1

Distribution: pure data-parallel, batch b -> core b (B=8, 8 cores).

Structure (all matmuls bf16 so the PE HAM clock stays at 2.4 GHz --
f32r/transpose-mode matmuls do not register as PE activity and leave the
array throttled at 1.2 GHz):

  * scores^T[l, hw] = M^T @ img with M = W1^T @ v: computed directly in
    the l-on-partitions orientation, so the softmax bias add and exp fuse
    into one ACT pass (bias is per-partition) and no transpose of the
    attention matrix is ever needed.
  * bf16x2 split precision for the scores chain (img = hi + lo,
    M = hi + lo; three cross terms) keeps scores at ~1e-4 relative error
    -- plain bf16 scores get amplified by the sharply peaked softmax.
  * softmax denominator comes free as a 65th row of the value matmul
    (vT augmented with a ones column); value is normalized after the
    matmul via a K=1 broadcast matmul of 1/denom.
  * conv3x3 = 9 shifted 1x1 convs over padded planes with row stride 65:
    col 0 of each row is zero and doubles as the right pad of the
    previous row, so each (tap, y-block) input window is one CONTIGUOUS
    [K, (r-1)*65+64] slice (matmul stationary operand must have a single
    free dim). Junk output columns (x=64) are dropped in the PSUM->SBUF
    copy. The attention value output lands directly in padded plane 2.
"""

import numpy as np
import ml_dtypes

import concourse.bass as bass
import concourse.tile as tile
from concourse import bacc, mybir
from concourse import bass_utils

F32 = mybir.dt.float32
BF16 = mybir.dt.bfloat16
BF = ml_dtypes.bfloat16

B = 8
CIN = 256  # img channels
CK = 64    # query/key channels
L = 256    # attention length
COUT = 256
H = W = 64
HW = H * W          # 4096
PS = W + 1          # 65: padded row stride
PH = H + 3          # 67 rows: top pad, 64 img rows, bottom pad, overrun row
NCORES = 8

# conv y-blocks: (start_row, nrows); PSUM free dim <= 512 limits to 7 rows
BLOCKS = [(7 * i, 7) for i in range(9)] + [(63, 1)]

# ---- v5 geometry: padded planes with stride 66 (4B-aligned rows) ----
# plane row layout: cols 0,1 = left pads, cols 2..65 = data x=0..63; the
# flattened next row's col 0 doubles as the right pad for tap dx=2.
PS6 = 66
PH6 = 67                  # top pad, 64 rows, bottom pad, overrun
PLANE = PH6 * PS6         # 4422
PLANE_A = PLANE + PS6     # 4488 allocated (zero tail for the +1/+66 shifts)
# value-channel tap pairing: pairs with partition-shift delta 1 (dx pairs)
# and delta 66 (one-row pair); tap8 stays single (K=64).
VPAIR_AB = [0, 3, 6]      # pairs (0,1), (3,4), (6,7) via the +1-shift plane
VPAIR_CD = [2]            # pair (2,5) via the +66-shift plane
# conv weight-chunk schedule: 18 img chunks + 3 AB pairs + 1 CD pair + tap8
NWCH = 23

_CACHE = {}
F32R = mybir.dt.float32r



def _build_nc_v5():
    """v5 family: PE-dense restructure of v4 (132.6us vs 287us measured v4).

    - 9 warmup matmuls + dummy exp at t=0 lift the HAM clock gate (needs
      ~3.4us of contiguous PE activity) and preload the ACT exp table while
      the input DMAs are still in flight; 6 more dummies after the M-phase
      bridge the first img-chunk DMA latency so scores start warm.
    - all DMAs contiguous (host-packed layouts); img streams in 512-pixel
      chunks through a 3-slot tile pool whose slot-reuse waits pace the
      DMA issue; padded conv planes are filled on-chip by DVE 4x copies.
    - attention is software-pipelined: scores/exp for chunk j, value+recip
      chain for j-1, 1/den broadcast (K=1 matmul) + normalize-multiply for
      j-3 -- the PE FIFO never waits on the DVE/ACT chains.  1/den uses
      reciprocal_approx_fast (DVE custom op, 5x cheaper than reciprocal);
      the den row is staged PSUM->SBUF on the ACT queue first (the approx
      op mis-reads large f32 directly from PSUM), and the bf16 cast rides
      the ACT queue too, keeping the DVE under the PE's per-chunk pace.
    - conv planes use row stride 66 (rows 4B-aligned -> 4x DVE fills); the
      9 K=64 value-channel taps become 4 K=128 pairs + 1 zero-padded K=128
      single via two partition-shifted copies of the value plane, so 27
      matmuls per output tile become 23, all with fast weight load.  The
      shifted copies are SBUF->SBUF DMAs split into block-pair-aligned row
      ranges so they stream during attention (they serialize on one HW DMA
      queue); conv weights stay stationary across y-block pairs; the first
      two block-pairs' img matmuls are emitted before the tail normalizes
      to cover the softmax-chain latency (all 1/den broadcast matmuls must
      precede any conv value-tap matmul or the PE FIFO deadlocks); one
      shared 8-bank PSUM pool serves every phase.
    - output is bf16 (host casts back to f32): halves the output DMA.
    """
    nc = bacc.Bacc("TRN2", target_bir_lowering=False, debug=False)

    imgh_d = nc.dram_tensor("img_hi", (CIN, HW), BF16, kind="ExternalInput")
    imgl_d = nc.dram_tensor("img_lo", (CIN, HW), BF16, kind="ExternalInput")
    v_d = nc.dram_tensor("v2p", (CK, 2, L), BF16, kind="ExternalInput")      # [k, hi/lo, l]
    vta_d = nc.dram_tensor("vtap", (128, 2, CK + 1), BF16, kind="ExternalInput")
    w1_d = nc.dram_tensor("w12p", (CK, 2, CIN), BF16, kind="ExternalInput")  # [k, hi/lo, c]
    b1_d = nc.dram_tensor("b1", (CK, 1), BF16, kind="ExternalInput")
    one_d = nc.dram_tensor("one64", (1, CK), BF16, kind="ExternalInput")
    w2_d = nc.dram_tensor("w2p23", (128, NWCH, COUT), BF16, kind="ExternalInput")
    b2_d = nc.dram_tensor("b2p", (128, 2, 1), F32, kind="ExternalInput")
    out_d = nc.dram_tensor("out", (COUT, HW), BF16, kind="ExternalOutput")

    with tile.TileContext(nc) as tc:
        with (
            tc.tile_pool(name="singles", bufs=1) as singles,
            tc.tile_pool(name="sm", bufs=3) as sm,
            tc.tile_pool(name="imp", bufs=3) as imp,
            tc.tile_pool(name="outp", bufs=3) as outp,
            tc.tile_pool(name="ps", bufs=8, space="PSUM") as ps_pool,
        ):
            def ps_tile(name):
                return ps_pool.tile([128, 512], F32, tag="ps", name=name,
                                    uniquify=True)

            # ---- resident tensors ----
            pc0 = singles.tile([128, PH6, PS6], BF16)
            pc1 = singles.tile([128, PH6, PS6], BF16)
            pcab = singles.tile([128, PH6 + 1, PS6], BF16)  # [V ; V shifted +1]
            pccd = singles.tile([128, PH6 + 1, PS6], BF16)  # [V ; V shifted +66]
            w2sb = singles.tile([128, NWCH, COUT], BF16)
            vta_sb = singles.tile([128, 2, CK + 1], BF16)
            v_sb = singles.tile([CK, 2, L], BF16)
            w1_sb = singles.tile([CK, 2, CIN], BF16)
            b1_sb = singles.tile([CK, 1], BF16)
            one_sb = singles.tile([1, CK], BF16)
            b2_sb = singles.tile([128, 2, 1], F32)
            m_sb = singles.tile([128, 2, 2, L], BF16)       # [cc, hi/lo, l]
            bcol_sb = singles.tile([128, 2, 1], F32)        # softmax bias per l-tile
            wtile = singles.tile([128, 512], BF16)
            dexp = singles.tile([1, 1], BF16)
            vtmpb = singles.tile([CK, 8, 512], BF16)        # unnormalized value
            denf = singles.tile([1, 8, 512], F32)           # den staged to SBUF
            rdenf = singles.tile([1, 8, 512], F32)          # 1/den (fp32)
            rdenb = singles.tile([1, 8, 512], BF16)

            fab = pcab[:].rearrange("p a b -> p (a b)")
            fcd = pccd[:].rearrange("p a b -> p (a b)")

            # ---- t=0: param DMAs, PE warmup, ACT table preload ----
            nc.scalar.dma_start(v_sb[:], v_d[:])
            nc.scalar.dma_start(w1_sb[:], w1_d[:])
            nc.scalar.dma_start(b1_sb[:], b1_d[:])
            nc.scalar.dma_start(one_sb[:], one_d[:])
            nc.scalar.dma_start(b2_sb[:], b2_d[:])
            nc.scalar.dma_start(vta_sb[:], vta_d[:])
            nc.vector.memset(wtile[:], 0.0)
            # pcab pads early (DVE is idle): the shifted-plane DMAs read them
            nc.vector.memset(pcab[0:64, 0, :], 0.0)
            nc.vector.memset(pcab[0:64, H + 1:PH6 + 1, :], 0.0)  # rows 65..67
            nc.vector.memset(pcab[0:64, 1:H + 1, 0:2], 0.0)
            nc.vector.memset(fab[64:128, PLANE_A - 1:PLANE_A], 0.0)
            psw = ps_tile("ps_warm")
            for _ in range(9):
                nc.tensor.matmul(psw[0:64, :], wtile[:, 0:64], wtile[:],
                                 start=True, stop=True)
            nc.scalar.activation(dexp[:], wtile[0:1, 0:1],
                                 mybir.ActivationFunctionType.Exp)


            # ---- M = W1^T @ v (bf16x2) ----
            for cc in range(2):
                psm = ps_tile("ps_m")
                w1s = w1_sb[:, :, cc * 128:(cc + 1) * 128]
                nc.tensor.matmul(psm[:, 0:L], w1s[:, 0, :], v_sb[:, 0, :], start=True, stop=False)
                nc.tensor.matmul(psm[:, 0:L], w1s[:, 0, :], v_sb[:, 1, :], start=False, stop=False)
                nc.tensor.matmul(psm[:, 0:L], w1s[:, 1, :], v_sb[:, 0, :], start=False, stop=True)
                nc.vector.tensor_copy(m_sb[:, cc, 0, :], psm[:, 0:L])
                nc.vector.tensor_tensor(
                    m_sb[:, cc, 1, :], psm[:, 0:L], m_sb[:, cc, 0, :],
                    mybir.AluOpType.subtract,
                )

            # ---- softmax bias column: bias[l] = sum_k b1[k] v[k, l] ----
            for lt in range(2):
                psb = ps_tile("ps_bias")
                vs = v_sb[:, :, lt * 128:(lt + 1) * 128]
                nc.tensor.matmul(psb[:, 0:1], vs[:, 0, :], b1_sb[:], start=True, stop=False)
                nc.tensor.matmul(psb[:, 0:1], vs[:, 1, :], b1_sb[:], start=False, stop=True)
                nc.vector.tensor_copy(bcol_sb[:, lt, :], psb[:, 0:1])

            # keep the PE (and HAM) busy while the first img chunk lands
            for _ in range(6):
                nc.tensor.matmul(psw[0:64, :], wtile[:, 0:64], wtile[:],
                                 start=True, stop=True)

            # ---- attention: scores/exp pipelined one chunk ahead of value ----
            expT = {}
            imtiles = {}

            def finish(i):
                psv = ps_tile("ps_v")
                for lt in range(2):
                    nc.tensor.matmul(
                        psv[0:CK + 1, :], vta_sb[:, lt, :], expT[(i, lt)][:],
                        start=(lt == 0), stop=(lt == 1),
                    )
                nc.vector.tensor_copy(vtmpb[:, i, :], psv[0:CK, :])
                if _CACHE.get("use_plain_recip"):
                    with nc.allow_low_precision(reason="1/denom via bf16"):
                        nc.vector.reciprocal(rdenb[:, i, :], psv[CK:CK + 1, :])
                else:
                    # approx_fast mis-reads large f32 straight from PSUM
                    # (bitwise seed path); stage den to SBUF first.  The two
                    # copies ride the half-idle ACT queue to keep the DVE
                    # chain under the PE's per-chunk pace.
                    nc.scalar.copy(denf[:, i, :], psv[CK:CK + 1, :])
                    nc.vector.reciprocal_approx_fast(rdenf[:, i, :], denf[:, i, :])
                    nc.scalar.copy(rdenb[:, i, :], rdenf[:, i, :])

            def normalize(i):
                psr = ps_tile("ps_r")
                nc.tensor.matmul(psr[0:CK, :], one_sb[:], rdenb[:, i, :],
                                 start=True, stop=True)
                nc.vector.tensor_tensor(
                    pcab[0:CK, 1 + i * 8: 9 + i * 8, 2:PS6],
                    vtmpb[:, i, :], psr[0:CK, :],
                    mybir.AluOpType.mult,
                )

            for j in range(8):
                hw = slice(j * 512, (j + 1) * 512)
                imt = imp.tile([128, 2, 2, 512], BF16, tag="imgc", name="imt")
                imtiles[j] = imt
                for cc in range(2):
                    rows = slice(cc * 128, (cc + 1) * 128)
                    nc.sync.dma_start(imt[:, cc, 0, :], imgh_d[rows, hw])
                    nc.gpsimd.dma_start(imt[:, cc, 1, :], imgl_d[rows, hw])
                for lt in range(2):
                    pst = ps_tile("ps_t")
                    k = 0
                    for cc in range(2):
                        ms = m_sb[:, cc, :, lt * 128:(lt + 1) * 128]
                        for (mh, ih) in ((0, 0), (0, 1), (1, 0)):
                            nc.tensor.matmul(
                                pst[:], ms[:, mh, :], imt[:, cc, ih, :],
                                start=(k == 0), stop=(k == 5),
                            )
                            k += 1
                    et = sm.tile([128, 512], BF16, tag=f"expT{lt}", name=f"expT{lt}")
                    nc.scalar.activation(
                        et[:], pst[:], mybir.ActivationFunctionType.Exp,
                        bias=bcol_sb[:, lt, :],
                    )
                    expT[(j, lt)] = et
                if j == 1:
                    # paced: fires on the ACT queue after chunk 1's exps,
                    # long before the conv needs the weights
                    nc.scalar.dma_start(w2sb[:], w2_d[:])
                if j == 3:
                    # img plane pads: after the early img-lo triggers (so they
                    # don't delay chunk DMAs) but well before the conv reads
                    for p in (pc0, pc1):
                        nc.gpsimd.memset(p[:, 0, :], 0.0)
                        nc.gpsimd.memset(p[:, H + 1, :], 0.0)
                        nc.gpsimd.memset(p[:, H + 2, :], 0.0)
                        nc.gpsimd.memset(p[:, 1:H + 1, 0:2], 0.0)
                if j > 0:
                    finish(j - 1)
                if j > 2:
                    normalize(j - 3)
                # fill conv img planes for this chunk (rows 8j+1 .. 8j+8);
                # emitted after the normalize chain so the DVE prioritizes it
                for cc in range(2):
                    nc.vector.tensor_copy(
                        [pc0, pc1][cc][:, 1 + j * 8: 9 + j * 8, 2:PS6],
                        imt[:, cc, 0, :],
                    )
            finish(7)


            # ---- 3x3 conv schedule ----
            pf0 = pc0[:].rearrange("p a b -> p (a b)")
            pf1 = pc1[:].rearrange("p a b -> p (a b)")
            wsched = []
            for t in range(9):
                for c in range(2):
                    wsched.append((128, [pf0, pf1][c], t // 3, t % 3))
            for t0 in VPAIR_AB:
                wsched.append((128, fab, t0 // 3, t0 % 3))
            wsched.append((128, fcd, 0, 2))   # pair (2, 5)
            wsched.append((128, fab, 2, 2))   # tap 8 (weight rows 64..127 zero)
            assert len(wsched) == NWCH

            def conv_pair(ot, bp, pscs, w_lo, w_hi, drain):
                ocols = slice(ot * 128, (ot + 1) * 128)
                blks = BLOCKS[2 * bp: 2 * bp + 2]
                for w in range(w_lo, w_hi):
                    kk, src, dy, dx = wsched[w]
                    lhsT = w2sb[0:kk, w, ocols]
                    for bi, (y0, r) in enumerate(blks):
                        n = (r - 1) * PS6 + W
                        base = (y0 + dy) * PS6 + dx + 1
                        nc.tensor.matmul(
                            pscs[bi][:, 0:n], lhsT, src[0:kk, base:base + n],
                            start=(w == 0), stop=(w == NWCH - 1),
                        )
                if drain:
                    last = (ot == 1 and bp == 4)
                    for bi, (y0, r) in enumerate(blks):
                        outt = outp.tile([128, r, W], BF16, tag="outt", name="outt")
                        srcv = pscs[bi].rearrange("p (a b) -> p a b", b=PS6)[:, 0:r, 0:W]
                        if last and bi == 1:
                            # final tile: DVE drain + sync-queue DMA run in
                            # parallel with the ACT drain of its sibling
                            nc.vector.tensor_scalar_add(
                                outt[:], srcv, b2_sb[:, ot, :])
                            nc.sync.dma_start(
                                out_d[ocols, y0 * W:(y0 + r) * W], outt[:])
                        else:
                            nc.scalar.activation(
                                outt[:], srcv,
                                mybir.ActivationFunctionType.Identity,
                                bias=b2_sb[:, ot, :],
                            )
                            (nc.sync if last else nc.gpsimd).dma_start(
                                out_d[ocols, y0 * W:(y0 + r) * W], outt[:],
                            )

            def conv_pscs(ot, bp):
                return [ps_pool.tile([128, 7 * PS6], F32, tag="ps",
                                     name=f"psc{ot}_{bp}_{bi}", uniquify=True)
                        for bi in range(2)]

            # first block-pair's img matmuls cover the tail normalize latency;
            # all psr matmuls MUST precede any conv value-tap matmul (the
            # value taps wait on mult(7) -> psr(7): emitting psr later would
            # deadlock the PE FIFO)
            pscs00 = conv_pscs(0, 0)
            conv_pair(0, 0, pscs00, 0, 18, drain=False)
            for j in (5, 6, 7):
                normalize(j)

            # ---- shifted value-plane copies (partition halves via DMA) ----
            # split into block-pair-aligned row ranges: subtile deps let each
            # piece fire as soon as its source rows are normalized, so the
            # (serialized) SBUF->SBUF DMA streams during the attention loop
            cuts = [0, 1056, 1980, 2904, 3828, PLANE_A]
            for a, b in zip(cuts[:-1], cuts[1:]):
                nc.sync.dma_start(fab[64:128, a:min(b, PLANE_A - 1)],
                                  fab[0:64, a + 1:min(b + 1, PLANE_A)])
                bc = min(b, PLANE)
                if a < bc:
                    nc.scalar.dma_start(fcd[0:64, a:bc], fab[0:64, a:bc])
                bh = min(b, PLANE_A - PS6)
                if a < bh:
                    nc.gpsimd.dma_start(fcd[64:128, a:bh],
                                        fab[0:64, a + PS6:bh + PS6])

            pscs01 = conv_pscs(0, 1)
            conv_pair(0, 1, pscs01, 0, 18, drain=False)
            conv_pair(0, 0, pscs00, 18, NWCH, drain=True)
            conv_pair(0, 1, pscs01, 18, NWCH, drain=True)
            for bp in range(2, 5):
                conv_pair(0, bp, conv_pscs(0, bp), 0, NWCH, drain=True)
            for bp in range(5):
                conv_pair(1, bp, conv_pscs(1, bp), 0, NWCH, drain=True)

    nc.compile()
    return nc


def _prep_in_maps_v5(img_embedding, v_embedding, W1, b1, W2, b2):
    # host-side layout prep (no math beyond dtype cast / transpose / pack)
    w2t = np.ascontiguousarray(
        W2.transpose(2, 3, 1, 0).reshape(9, CIN + CK, COUT).astype(np.float32)
    )
    w2p = np.zeros((128, NWCH, COUT), BF)
    for t in range(9):
        w2p[:, 2 * t + 0, :] = w2t[t, 0:128, :].astype(BF)
        w2p[:, 2 * t + 1, :] = w2t[t, 128:256, :].astype(BF)
    for i, t0 in enumerate(VPAIR_AB):
        w2p[0:64, 18 + i, :] = w2t[t0, 256:320, :].astype(BF)
        w2p[64:128, 18 + i, :] = w2t[t0 + 1, 256:320, :].astype(BF)
    w2p[0:64, 21, :] = w2t[2, 256:320, :].astype(BF)
    w2p[64:128, 21, :] = w2t[5, 256:320, :].astype(BF)
    w2p[0:64, 22, :] = w2t[8, 256:320, :].astype(BF)

    w1h, w1l = _split_bf16x2(np.asarray(W1, np.float32))
    w12 = np.stack([w1h, w1l], axis=1)          # [64, 2, 256]
    b1f = np.asarray(b1, np.float32).reshape(CK, 1).astype(BF)
    one64 = np.ones((1, CK), BF)
    b2f = np.ascontiguousarray(
        np.asarray(b2, np.float32).reshape(2, 128).transpose(1, 0).reshape(128, 2, 1)
    )

    in_maps = []
    for bb in range(B):
        img = np.asarray(img_embedding[bb], np.float32).reshape(CIN, HW)
        ih, il = _split_bf16x2(img)
        v32 = np.asarray(v_embedding[bb], np.float32)
        vh, vl = _split_bf16x2(v32)
        v2p = np.stack([vh, vl], axis=1)        # [64, 2, 256]
        vta = np.ones((L, CK + 1), BF)
        vta[:, 0:CK] = v32.T.astype(BF)
        vtap = np.ascontiguousarray(
            vta.reshape(2, 128, CK + 1).transpose(1, 0, 2)
        )                                        # [128, 2, 65]
        in_maps.append(
            {
                "img_hi": np.ascontiguousarray(ih),
                "img_lo": np.ascontiguousarray(il),
                "v2p": np.ascontiguousarray(v2p),
                "vtap": vtap,
                "w12p": np.ascontiguousarray(w12),
                "b1": b1f,
                "one64": one64,
                "w2p23": np.ascontiguousarray(w2p),
                "b2p": b2f,
            }
        )
    return in_maps


def _build_nc_v4():
    nc = bacc.Bacc("TRN2", target_bir_lowering=False, debug=False)

    imgh_d = nc.dram_tensor("img_hi", (CIN, HW), BF16, kind="ExternalInput")
    imgl_d = nc.dram_tensor("img_lo", (CIN, HW), BF16, kind="ExternalInput")
    v_d = nc.dram_tensor("v2", (2, CK, L), BF16, kind="ExternalInput")     # hi, lo
    vta_d = nc.dram_tensor("vta", (L, CK + 1), BF16, kind="ExternalInput")  # v^T | 1
    w1_d = nc.dram_tensor("w12", (2, CK, CIN), BF16, kind="ExternalInput")  # hi, lo
    b1_d = nc.dram_tensor("b1", (CK, 1), BF16, kind="ExternalInput")
    one_d = nc.dram_tensor("one64", (1, CK), BF16, kind="ExternalInput")
    w2_d = nc.dram_tensor("w2p", (128, 27, COUT), BF16, kind="ExternalInput")
    b2_d = nc.dram_tensor("b2", (COUT, 1), F32, kind="ExternalInput")
    out_d = nc.dram_tensor("out", (COUT, HW), F32, kind="ExternalOutput")

    with tile.TileContext(nc) as tc:
        with (
            tc.tile_pool(name="singles", bufs=1) as singles,
            tc.tile_pool(name="sm", bufs=4) as sm,
            tc.tile_pool(name="outp", bufs=3) as outp,
            tc.tile_pool(name="ps_t", bufs=2, space="PSUM") as ps_t,
            tc.tile_pool(name="ps_v", bufs=3, space="PSUM") as ps_v,
            tc.tile_pool(name="ps_c", bufs=2, space="PSUM") as ps_c,
        ):
            # ---- resident tensors ----
            pc0 = singles.tile([128, PH, PS], BF16)
            pc1 = singles.tile([128, PH, PS], BF16)
            pc2 = singles.tile([CK, PH, PS], BF16)
            pci = [pc0, pc1]
            imgc = singles.tile([128, 2, 2, HW], BF16)  # [cc, hi/lo, hw]
            w2sb = singles.tile([128, 27, COUT], BF16)
            vta_sb = singles.tile([128, 2, CK + 1], BF16)
            v_sb = singles.tile([CK, 2, L], BF16)
            w1_sb = singles.tile([CK, 2, CIN], BF16)
            b1_sb = singles.tile([CK, 1], BF16)
            one_sb = singles.tile([1, CK], BF16)
            b2_sb = singles.tile([128, 2, 1], F32)
            m_sb = singles.tile([128, 2, 2, L], BF16)   # [cc, hi/lo, l]
            bcol_sb = singles.tile([128, 2, 1], F32)    # softmax bias, per l-tile

            # ---- small input DMAs on the scalar queue (scores path first) ----
            nc.scalar.dma_start(v_sb[:], v_d.rearrange("h k l -> k h l"))
            nc.scalar.dma_start(w1_sb[:], w1_d.rearrange("h k c -> k h c"))
            nc.scalar.dma_start(b1_sb[:], b1_d[:])
            nc.scalar.dma_start(one_sb[:], one_d[:])
            nc.scalar.dma_start(b2_sb[:], b2_d.rearrange("(t p) x -> p t x", p=128))
            nc.scalar.dma_start(vta_sb[:], vta_d.rearrange("(lc p) c -> p lc c", p=128))
            for cc in range(2):
                nc.scalar.dma_start(imgc[:, cc, 0, :], imgh_d[cc * 128:(cc + 1) * 128, :])
                nc.scalar.dma_start(imgc[:, cc, 1, :], imgl_d[cc * 128:(cc + 1) * 128, :])

            # ---- bulk input DMAs on the sync queue ----
            for p in (pc0, pc1, pc2):
                nc.vector.memset(p[:, 0, :], 0.0)        # top pad row
                nc.vector.memset(p[:, H + 1, :], 0.0)    # bottom pad row
                nc.vector.memset(p[:, H + 2, :], 0.0)    # overrun row
                nc.vector.memset(p[:, 1:H + 1, 0:1], 0.0)  # left pad col (= right pad)
            for cc in range(2):
                nc.sync.dma_start(
                    pci[cc][:, 1:H + 1, 1:PS],
                    imgh_d[cc * 128:(cc + 1) * 128, :].rearrange("p (h w) -> p h w", w=W),
                )
            nc.sync.dma_start(w2sb[:], w2_d[:])

            # ---- M = W1^T @ v (bf16x2), split into hi/lo planes ----
            for cc in range(2):
                ps = ps_t.tile([128, 512], F32, tag="pst", name="ps_m")
                w1s = w1_sb[:, :, cc * 128:(cc + 1) * 128]
                nc.tensor.matmul(ps[:, 0:L], w1s[:, 0, :], v_sb[:, 0, :], start=True, stop=False)
                nc.tensor.matmul(ps[:, 0:L], w1s[:, 0, :], v_sb[:, 1, :], start=False, stop=False)
                nc.tensor.matmul(ps[:, 0:L], w1s[:, 1, :], v_sb[:, 0, :], start=False, stop=True)
                nc.vector.tensor_copy(m_sb[:, cc, 0, :], ps[:, 0:L])
                nc.vector.tensor_tensor(
                    m_sb[:, cc, 1, :], ps[:, 0:L], m_sb[:, cc, 0, :],
                    mybir.AluOpType.subtract,
                )

            # ---- softmax bias column: bias[l] = sum_k b1[k] v[k, l] ----
            for lt in range(2):
                psc0 = ps_v.tile([128, 512], F32, tag="psv", name="ps_bias")
                vs = v_sb[:, :, lt * 128:(lt + 1) * 128]
                nc.tensor.matmul(psc0[:, 0:1], vs[:, 0, :], b1_sb[:], start=True, stop=False)
                nc.tensor.matmul(psc0[:, 0:1], vs[:, 1, :], b1_sb[:], start=False, stop=True)
                nc.vector.tensor_copy(bcol_sb[:, lt, :], psc0[:, 0:1])

            # ---- attention, per 512-pixel chunk ----
            for j in range(8):
                hw = slice(j * 512, (j + 1) * 512)
                expT = []
                for lt in range(2):
                    pst = ps_t.tile([128, 512], F32, tag="pst")
                    k = 0
                    for cc in range(2):
                        ms = m_sb[:, cc, :, lt * 128:(lt + 1) * 128]
                        for (mh, ih) in ((0, 0), (0, 1), (1, 0)):
                            nc.tensor.matmul(
                                pst[:], ms[:, mh, :], imgc[:, cc, ih, hw],
                                start=(k == 0), stop=(k == 5),
                            )
                            k += 1
                    # exp(scores + b1@v) with the bias fused as per-partition ACT bias
                    et = sm.tile([128, 512], BF16, tag=f"expT{lt}", name=f"expT{lt}")
                    nc.scalar.activation(
                        et[:], pst[:], mybir.ActivationFunctionType.Exp,
                        bias=bcol_sb[:, lt, :],
                    )
                    expT.append(et)
                # value (rows 0:64) + softmax denominator (row 64)
                psv = ps_v.tile([CK + 1, 512], F32, tag="psv", name="psv")
                for lt in range(2):
                    nc.tensor.matmul(
                        psv[:], vta_sb[:, lt, :], expT[lt][:],
                        start=(lt == 0), stop=(lt == 1),
                    )
                rden = sm.tile([1, 512], BF16, tag="rden")
                with nc.allow_low_precision(reason="1/denom broadcast via bf16 matmul"):
                    nc.vector.reciprocal(rden[:], psv[CK:CK + 1, :])
                vtmp = sm.tile([CK, 512], F32, tag="vtmp")
                nc.vector.tensor_copy(vtmp[:], psv[0:CK, :])
                # broadcast 1/den across the 64 value partitions via K=1 matmul
                psr = ps_v.tile([CK, 512], F32, tag="psv", name="psr")
                nc.tensor.matmul(psr[:], one_sb[:], rden[:], start=True, stop=True)
                nc.vector.tensor_tensor(
                    pc2[:, 1 + j * 8: 9 + j * 8, 1:PS], vtmp[:], psr[:],
                    mybir.AluOpType.mult,
                )

            # ---- 3x3 conv: 9 shifted matmuls x 3 channel chunks ----
            pf = [p[:].rearrange("p a b -> p (a b)") for p in (pc0, pc1, pc2)]
            for ot in range(2):
                for y0, r in BLOCKS:
                    n = (r - 1) * PS + W  # contiguous window length
                    psc = ps_c.tile([128, 7 * PS], F32)
                    k = 0
                    for tap in range(9):
                        dy, dx = tap // 3, tap % 3
                        base = (y0 + dy) * PS + dx
                        for c in range(3):
                            kk = 128 if c < 2 else CK
                            lhsT = w2sb[0:kk, tap * 3 + c, ot * 128:(ot + 1) * 128]
                            nc.tensor.matmul(
                                psc[:, 0:n], lhsT, pf[c][0:kk, base:base + n],
                                start=(k == 0), stop=(k == 26),
                            )
                            k += 1
                    outt = outp.tile([128, r, W], F32, tag="outt")
                    src = psc.rearrange("p (a b) -> p a b", b=PS)[:, 0:r, 0:W]
                    nc.scalar.activation(
                        outt[:], src, mybir.ActivationFunctionType.Identity,
                        bias=b2_sb[:, ot, :],
                    )
                    nc.sync.dma_start(
                        out_d[ot * 128:(ot + 1) * 128, y0 * W:(y0 + r) * W],
                        outt[:],
                    )

    nc.compile()
    return nc


def _prep_in_maps_v4(img_embedding, v_embedding, W1, b1, W2, b2):
    # host-side layout prep (no math beyond dtype cast / transpose / pack)
    w2t = np.ascontiguousarray(
        W2.transpose(2, 3, 1, 0).reshape(9, CIN + CK, COUT).astype(np.float32)
    )
    w2p = np.zeros((128, 27, COUT), BF)
    for t in range(9):
        w2p[:, t * 3 + 0, :] = w2t[t, 0:128, :].astype(BF)
        w2p[:, t * 3 + 1, :] = w2t[t, 128:256, :].astype(BF)
        w2p[0:CK, t * 3 + 2, :] = w2t[t, 256:320, :].astype(BF)
    w1h, w1l = _split_bf16x2(np.asarray(W1, np.float32))
    w12 = np.stack([w1h, w1l])
    b1f = np.asarray(b1, np.float32).reshape(CK, 1).astype(BF)
    one64 = np.ones((1, CK), BF)
    b2f = np.ascontiguousarray(np.asarray(b2, np.float32).reshape(COUT, 1))

    in_maps = []
    for bb in range(B):
        img = np.asarray(img_embedding[bb], np.float32).reshape(CIN, HW)
        ih, il = _split_bf16x2(img)
        v32 = np.asarray(v_embedding[bb], np.float32)
        vh, vl = _split_bf16x2(v32)
        vta = np.ones((L, CK + 1), BF)
        vta[:, 0:CK] = v32.T.astype(BF)
        in_maps.append(
            {
                "img_hi": np.ascontiguousarray(ih),
                "img_lo": np.ascontiguousarray(il),
                "v2": np.stack([vh, vl]),
                "vta": vta,
                "w12": w12,
                "b1": b1f,
                "one64": one64,
                "w2p": w2p,
                "b2": b2f,
            }
        )
    return in_maps


def _split_bf16x2(a):
    hi = a.astype(BF)
    lo = (a - hi.astype(np.float32)).astype(BF)
    return hi, lo


def _round_f32r(a):
    """Round-to-nearest-even fp32 -> fp32r (11-bit mantissa, low 12 bits zero)."""
    u = np.ascontiguousarray(a, dtype=np.float32).view(np.uint32)
    u = (u + 0x7FF + ((u >> 12) & 1)) & np.uint32(0xFFFFF000)
    return u.view(np.float32)


def _build_nc_v3():
    nc = bacc.Bacc("TRN2", target_bir_lowering=False, debug=False)

    img_d = nc.dram_tensor("img", (CIN, HW), F32R, kind="ExternalInput")
    v_d = nc.dram_tensor("v", (CK, L), F32R, kind="ExternalInput")
    vt_d = nc.dram_tensor("vt_bf", (L, CK), BF16, kind="ExternalInput")
    w1_d = nc.dram_tensor("w1", (CK, CIN), F32R, kind="ExternalInput")
    b1_d = nc.dram_tensor("b1p", (CK, 128), F32R, kind="ExternalInput")
    w2_d = nc.dram_tensor("w2p", (128, 18, COUT), F32R, kind="ExternalInput")
    w2v_d = nc.dram_tensor("w2v", (CK, 9, COUT), BF16, kind="ExternalInput")
    b2_d = nc.dram_tensor("b2", (COUT, 1), F32, kind="ExternalInput")
    zz_d = nc.dram_tensor("zz", (128, PS), F32R, kind="ExternalInput")
    out_d = nc.dram_tensor("out", (COUT, HW), F32, kind="ExternalOutput")

    with tile.TileContext(nc) as tc:
        with (
            tc.tile_pool(name="singles", bufs=1) as singles,
            tc.tile_pool(name="sm", bufs=3) as sm,
            tc.tile_pool(name="outp", bufs=3) as outp,
            tc.tile_pool(name="ps_s", bufs=2, space="PSUM") as ps_s,
            tc.tile_pool(name="ps_v", bufs=2, space="PSUM") as ps_v,
            tc.tile_pool(name="ps_c", bufs=2, space="PSUM") as ps_c,
        ):
            # ---- resident tensors ----
            pc0 = singles.tile([128, PH, PS], F32R)
            pc1 = singles.tile([128, PH, PS], F32R)
            pc2 = singles.tile([CK, PH, PS], BF16)
            pci = [pc0, pc1]
            imgc = singles.tile([128, 2, HW], F32R)  # contiguous img, scores lhsT
            w2sb = singles.tile([128, 18, COUT], F32R)
            w2v_sb = singles.tile([CK, 9, COUT], BF16)
            vt_sb = singles.tile([128, 2, CK], BF16)
            v_sb = singles.tile([CK, L], F32R)
            w1_sb = singles.tile([CK, CIN], F32R)
            b1_sb = singles.tile([CK, 128], F32R)
            b2_sb = singles.tile([128, 2, 1], F32)
            m_sb = singles.tile([128, 2, L], F32R)
            bias_bc = singles.tile([128, L], F32)
            attT = [
                singles.tile([128, HW], BF16, tag=f"attT{lc}", name=f"attT{lc}")
                for lc in range(2)
            ]

            # ---- input DMAs + pad-zeroing ----
            # (DVE memset on float32r is an invalid ISA encoding -- zero the
            # f32r plane pads by DMA from a zeros DRAM tensor instead)
            for p in (pc0, pc1):
                nc.sync.dma_start(p[:, 0, :], zz_d[:])         # top pad row
                nc.sync.dma_start(p[:, H + 1, :], zz_d[:])     # bottom pad row
                nc.sync.dma_start(p[:, H + 2, :], zz_d[:])     # overrun row
                nc.sync.dma_start(p[:, 1:H + 1, 0:1], zz_d[:, 0:H].rearrange("p (w o) -> p w o", o=1))
            nc.vector.memset(pc2[:, 0, :], 0.0)
            nc.vector.memset(pc2[:, H + 1, :], 0.0)
            nc.vector.memset(pc2[:, H + 2, :], 0.0)
            nc.vector.memset(pc2[:, 1:H + 1, 0:1], 0.0)
            for c in range(2):
                nc.sync.dma_start(
                    pci[c][:, 1:H + 1, 1:PS],
                    img_d[c * 128:(c + 1) * 128, :].rearrange("p (h w) -> p h w", w=W),
                )
                nc.sync.dma_start(imgc[:, c, :], img_d[c * 128:(c + 1) * 128, :])

            nc.sync.dma_start(w2sb[:], w2_d[:])
            nc.sync.dma_start(w2v_sb[:], w2v_d[:])
            nc.sync.dma_start(vt_sb[:], vt_d.rearrange("(lc p) c -> p lc c", p=128))
            nc.sync.dma_start(v_sb[:], v_d[:])
            nc.sync.dma_start(w1_sb[:], w1_d[:])
            nc.sync.dma_start(b1_sb[:], b1_d[:])
            nc.sync.dma_start(b2_sb[:], b2_d.rearrange("(t p) x -> p t x", p=128))

            # ---- M = W1^T @ v  [Cin, L], bias broadcast [128, L] ----
            for cc in range(2):
                ps = ps_s.tile([128, L], F32, tag="scores", name="ps_m")
                nc.tensor.matmul(
                    ps[:], w1_sb[:, cc * 128:(cc + 1) * 128], v_sb[:],
                    start=True, stop=True,
                )
                nc.vector.tensor_copy(m_sb[:, cc, :], ps[:])
            # b1 is replicated across all 128 lhsT columns host-side, so this
            # matmul directly materializes bias_row broadcast over partitions
            psb = ps_s.tile([128, L], F32, tag="scores", name="psb")
            nc.tensor.matmul(psb[:], b1_sb[:], v_sb[:], start=True, stop=True)
            nc.vector.tensor_copy(bias_bc[:], psb[:])

            # ---- scores + softmax + transpose, per 128-pixel tile ----
            for i in range(HW // 128):
                ps = ps_s.tile([128, L], F32, tag="scores")
                for cc in range(2):
                    nc.tensor.matmul(
                        ps[:], imgc[:, cc, i * 128:(i + 1) * 128], m_sb[:, cc, :],
                        start=(cc == 0), stop=(cc == 1),
                    )
                nc.vector.tensor_add(ps[:], ps[:], bias_bc[:])
                exp_sb = sm.tile([128, L], F32, tag="exp")
                den = sm.tile([128, 1], F32, tag="den")
                nc.scalar.activation(
                    exp_sb[:], ps[:], mybir.ActivationFunctionType.Exp,
                    accum_out=den[:],
                )
                rden = sm.tile([128, 1], F32, tag="rden")
                nc.vector.reciprocal(rden[:], den[:])
                att = sm.tile([128, L], BF16, tag="att")
                nc.vector.tensor_scalar_mul(att[:], exp_sb[:], rden[:])
                for lc in range(2):
                    nc.sync.dma_start(
                        attT[lc][:, i * 128:(i + 1) * 128],
                        att[:, lc * 128:(lc + 1) * 128],
                        transpose=True,
                    )

            # ---- value = v @ att^T, written into padded plane 2 ----
            for j in range(8):
                psv = ps_v.tile([CK, 8, W], F32)
                for lc in range(2):
                    nc.tensor.matmul(
                        psv[:], vt_sb[:, lc, :], attT[lc][:, j * 512:(j + 1) * 512],
                        start=(lc == 0), stop=(lc == 1),
                    )
                nc.vector.tensor_copy(pc2[:, 1 + j * 8: 9 + j * 8, 1:PS], psv[:])

            # ---- 3x3 conv: 9 shifted matmuls x 3 channel chunks ----
            pf = [p[:].rearrange("p a b -> p (a b)") for p in (pc0, pc1, pc2)]
            for ot in range(2):
                for y0, r in BLOCKS:
                    n = (r - 1) * PS + W  # contiguous window length
                    psc = ps_c.tile([128, 7 * PS], F32)
                    k = 0
                    for tap in range(9):
                        dy, dx = tap // 3, tap % 3
                        base = (y0 + dy) * PS + dx
                        for c in range(3):
                            if c < 2:
                                lhsT = w2sb[:, tap * 2 + c, ot * 128:(ot + 1) * 128]
                            else:
                                lhsT = w2v_sb[:, tap, ot * 128:(ot + 1) * 128]
                            nc.tensor.matmul(
                                psc[:, 0:n], lhsT, pf[c][0:(128 if c < 2 else CK), base:base + n],
                                start=(k == 0), stop=(k == 26),
                            )
                            k += 1
                    outt = outp.tile([128, r, W], F32, tag="outt")
                    src = psc.rearrange("p (a b) -> p a b", b=PS)[:, 0:r, 0:W]
                    nc.scalar.activation(
                        outt[:], src, mybir.ActivationFunctionType.Identity,
                        bias=b2_sb[:, ot, :],
                    )
                    nc.sync.dma_start(
                        out_d[ot * 128:(ot + 1) * 128, y0 * W:(y0 + r) * W],
                        outt[:],
                    )

    nc.compile()
    return nc


def _prep_in_maps_v3(img_embedding, v_embedding, W1, b1, W2, b2):
    # host-side layout prep (no math beyond dtype cast / transpose / pack)
    w2t = np.ascontiguousarray(
        W2.transpose(2, 3, 1, 0).reshape(9, CIN + CK, COUT).astype(np.float32)
    )
    w2p = np.zeros((128, 18, COUT), np.float32)
    for t in range(9):
        w2p[:, t * 2 + 0, :] = w2t[t, 0:128, :]
        w2p[:, t * 2 + 1, :] = w2t[t, 128:256, :]
    w2p = _round_f32r(w2p)
    w2v = np.ascontiguousarray(
        w2t[:, 256:320, :].transpose(1, 0, 2).astype(ml_dtypes.bfloat16)
    )
    w1f = _round_f32r(W1)
    b1p = np.repeat(np.asarray(b1, np.float32).reshape(CK, 1), 128, axis=1)
    b1p = _round_f32r(b1p)
    b2f = np.ascontiguousarray(np.asarray(b2, np.float32).reshape(COUT, 1))
    zz = np.zeros((128, PS), np.float32)

    in_maps = []
    for bb in range(B):
        img = _round_f32r(np.asarray(img_embedding[bb], np.float32).reshape(CIN, HW))
        v32 = np.asarray(v_embedding[bb], np.float32)
        v = _round_f32r(v32)
        vt = np.ascontiguousarray(v32.T.astype(ml_dtypes.bfloat16))
        in_maps.append(
            {
                "img": img,
                "v": v,
                "vt_bf": vt,
                "w1": w1f,
                "b1p": b1p,
                "w2p": w2p,
                "w2v": w2v,
                "b2": b2f,
                "zz": zz,
            }
        )
    return in_maps


def _run(build, prep, key, inputs, trace=False, **kw):
    import sys
    print(f"[kernel] path={key}", file=sys.stderr)
    if key not in _CACHE:
        _CACHE[key] = build()
    in_maps = prep(
        inputs["img_embedding"], inputs["v_embedding"],
        inputs["W1"], inputs["b1"], inputs["W2"], inputs["b2"],
    )
    return bass_utils.run_bass_kernel_spmd(
        _CACHE[key], in_maps, core_ids=list(range(NCORES)), trace=trace, **kw
    )


def run_spmd(inputs, trace=False, **kwargs):
    """v5 (PE-dense) with fallback to silicon-verified v4 then v3."""
    if not _CACHE.get("v5_bad"):
        try:
            return _run(_build_nc_v5, _prep_in_maps_v5, "v5", inputs, trace, **kwargs)
        except Exception:
            _CACHE["v5_bad"] = True
    if not _CACHE.get("v4_bad"):
        try:
            return _run(_build_nc_v4, _prep_in_maps_v4, "v4", inputs, trace, **kwargs)
        except Exception:
            _CACHE["v4_bad"] = True
    return _run(_build_nc_v3, _prep_in_maps_v3, "v3", inputs, trace, **kwargs)


def kernel(**inputs):
    res = run_spmd(inputs)
    out = np.stack([res.results[c]["out"] for c in range(NCORES)])
    return out.reshape(B, COUT, H, W).astype(np.float32)



# revision 26
# speedup vs baseline: 1.1890x; 1.1890x over previous
"""Trainium2 Bass kernel for nn_AttentionLayer (per-pixel attention + 3x3 conv).

Problem (per batch b):
    query = W1 @ img + b1                       # [Ck=64, HW]
    scores[hw, l] = sum_k query[k, hw] v[k, l]  # [HW, L=256]
    att = softmax(scores, axis=l)
    value[c, hw] = sum_l att[hw, l] v[c, l]     # [64, HW]
    cat = [img; value]                          # [320, HW]
    out = conv3x3(cat, W2) + b2                 # [256, H, W], padding=1

Distribution: pure data-parallel, batch b -> core b (B=8, 8 cores).

Structure (all matmuls bf16 so the PE HAM clock stays at 2.4 GHz --
f32r/transpose-mode matmuls do not register as PE activity and leave the
array throttled at 1.2 GHz):

  * scores^T[l, hw] = M^T @ img with M = W1^T @ v: computed directly in
    the l-on-partitions orientation, so the softmax bias add and exp fuse
    into one ACT pass (bias is per-partition) and no transpose of the
    attention matrix is ever needed.
  * bf16x2 split precision for the scores chain (img = hi + lo,
    M = hi + lo; three cross terms) keeps scores at ~1e-4 relative error
    -- plain bf16 scores get amplified by the sharply peaked softmax.
  * softmax denominator comes free as a 65th row of the value matmul
    (vT augmented with a ones column); value is normalized after the
    matmul via a K=1 broadcast matmul of 1/denom.
  * conv3x3 = 9 shifted 1x1 convs over padded planes with row stride 65:
    col 0 of each row is zero and doubles as the right pad of the
    previous row, so each (tap, y-block) input window is one CONTIGUOUS
    [K, (r-1)*65+64] slice (matmul stationary operand must have a single
    free dim). Junk output columns (x=64) are dropped in the PSUM->SBUF
    copy. The attention value output lands directly in padded plane 2.
"""

import numpy as np
import ml_dtypes

import concourse.bass as bass
import concourse.tile as tile
from concourse import bacc, mybir
from concourse import bass_utils

F32 = mybir.dt.float32
BF16 = mybir.dt.bfloat16
BF = ml_dtypes.bfloat16

B = 8
CIN = 256  # img channels
CK = 64    # query/key channels
L = 256    # attention length
COUT = 256
H = W = 64
HW = H * W          # 4096
PS = W + 1          # 65: padded row stride
PH = H + 3          # 67 rows: top pad, 64 img rows, bottom pad, overrun row
NCORES = 8

# conv y-blocks: (start_row, nrows); PSUM free dim <= 512 limits to 7 rows
BLOCKS = [(7 * i, 7) for i in range(9)] + [(63, 1)]

# ---- v5 geometry: padded planes with stride 66 (4B-aligned rows) ----
# plane row layout: cols 0,1 = left pads, cols 2..65 = data x=0..63; the
# flattened next row's col 0 doubles as the right pad for tap dx=2.
PS6 = 66
PH6 = 67                  # top pad, 64 rows, bottom pad, overrun
PLANE = PH6 * PS6         # 4422
PLANE_A = PLANE + PS6     # 4488 allocated (zero tail for the +1/+66 shifts)
# value-channel tap pairing: pairs with partition-shift delta 1 (dx pairs)
# and delta 66 (one-row pair); tap8 stays single (K=64).
VPAIR_AB = [0, 3, 6]      # pairs (0,1), (3,4), (6,7) via the +1-shift plane
VPAIR_CD = [2]            # pair (2,5) via the +66-shift plane
# conv weight-chunk schedule: 18 img chunks + 3 AB pairs + 1 CD pair + tap8
NWCH = 23

_CACHE = {}
F32R = mybir.dt.float32r



def _build_nc_v5():
    """v5 family: PE-dense restructure of v4 (132.6us vs 287us measured v4).

    - 9 warmup matmuls + dummy exp at t=0 lift the HAM clock gate (needs
      ~3.4us of contiguous PE activity) and preload the ACT exp table while
      the input DMAs are still in flight; 6 more dummies after the M-phase
      bridge the first img-chunk DMA latency so scores start warm.
    - all DMAs contiguous (host-packed layouts); img streams in 512-pixel
      chunks through a 3-slot tile pool whose slot-reuse waits pace the
      DMA issue; padded conv planes are filled on-chip by DVE 4x copies.
    - attention is software-pipelined: scores/exp for chunk j, value+recip
      chain for j-1, 1/den broadcast (K=1 matmul) + normalize-multiply for
      j-3 -- the PE FIFO never waits on the DVE/ACT chains.  1/den uses
      reciprocal_approx_fast (DVE custom op, 5x cheaper than reciprocal);
      the den row is staged PSUM->SBUF on the ACT queue first (the approx
      op mis-reads large f32 directly from PSUM), and the bf16 cast rides
      the ACT queue too, keeping the DVE under the PE's per-chunk pace.
    - conv planes use row stride 66 (rows 4B-aligned -> 4x DVE fills); the
      9 K=64 value-channel taps become 4 K=128 pairs + 1 zero-padded K=128
      single via two partition-shifted copies of the value plane, so 27
      matmuls per output tile become 23, all with fast weight load.  The
      shifted copies are SBUF->SBUF DMAs split into block-pair-aligned row
      ranges so they stream during attention (they serialize on one HW DMA
      queue); conv weights stay stationary across y-block pairs; the first
      two block-pairs' img matmuls are emitted before the tail normalizes
      to cover the softmax-chain latency (all 1/den broadcast matmuls must
      precede any conv value-tap matmul or the PE FIFO deadlocks); one
      shared 8-bank PSUM pool serves every phase.
    - output is bf16 (host casts back to f32): halves the output DMA.
    """
    nc = bacc.Bacc("TRN2", target_bir_lowering=False, debug=False)

    imgh_d = nc.dram_tensor("img_hi", (CIN, HW), BF16, kind="ExternalInput")
    imgl_d = nc.dram_tensor("img_lo", (CIN, HW), BF16, kind="ExternalInput")
    v_d = nc.dram_tensor("v2p", (CK, 2, L), BF16, kind="ExternalInput")      # [k, hi/lo, l]
    vta_d = nc.dram_tensor("vtap", (128, 2, CK + 1), BF16, kind="ExternalInput")
    w1_d = nc.dram_tensor("w12p", (CK, 2, CIN), BF16, kind="ExternalInput")  # [k, hi/lo, c]
    b1_d = nc.dram_tensor("b1", (CK, 1), BF16, kind="ExternalInput")
    one_d = nc.dram_tensor("one64", (1, CK), BF16, kind="ExternalInput")
    w2_d = nc.dram_tensor("w2p23", (128, NWCH, COUT), BF16, kind="ExternalInput")
    b2_d = nc.dram_tensor("b2p", (128, 2, 1), F32, kind="ExternalInput")
    out_d = nc.dram_tensor("out", (COUT, HW), BF16, kind="ExternalOutput")

    with tile.TileContext(nc) as tc:
        with (
            tc.tile_pool(name="singles", bufs=1) as singles,
            tc.tile_pool(name="sm", bufs=4) as sm,
            tc.tile_pool(name="imp", bufs=3) as imp,
            tc.tile_pool(name="outp", bufs=4) as outp,
            tc.tile_pool(name="ps", bufs=8, space="PSUM") as ps_pool,
        ):
            def ps_tile(name):
                return ps_pool.tile([128, 512], F32, tag="ps", name=name,
                                    uniquify=True)

            # ---- resident tensors ----
            pc0 = singles.tile([128, PH6, PS6], BF16)
            pc1 = singles.tile([128, PH6, PS6], BF16)
            pcab = singles.tile([128, PH6 + 1, PS6], BF16)  # [V ; V shifted +1]
            pccd = singles.tile([128, PH6 + 1, PS6], BF16)  # [V ; V shifted +66]
            w2sb = singles.tile([128, NWCH, COUT], BF16)
            vta_sb = singles.tile([128, 2, CK + 1], BF16)
            v_sb = singles.tile([CK, 2, L], BF16)
            w1_sb = singles.tile([CK, 2, CIN], BF16)
            b1_sb = singles.tile([CK, 1], BF16)
            one_sb = singles.tile([1, CK], BF16)
            b2_sb = singles.tile([128, 2, 1], F32)
            m_sb = singles.tile([128, 2, 2, L], BF16)       # [cc, hi/lo, l]
            bcol_sb = singles.tile([128, 2, 1], F32)        # softmax bias per l-tile
            wtile = singles.tile([128, 512], BF16)
            dexp = singles.tile([1, 1], BF16)
            vtmpb = singles.tile([CK, 8, 512], BF16)        # unnormalized value
            denf = singles.tile([1, 8, 512], F32)           # den staged to SBUF
            rdenf = singles.tile([1, 8, 512], F32)          # 1/den (fp32)
            rdenb = singles.tile([1, 8, 512], BF16)

            fab = pcab[:].rearrange("p a b -> p (a b)")
            fcd = pccd[:].rearrange("p a b -> p (a b)")

            # ---- t=0: param DMAs, PE warmup, ACT table preload ----
            nc.scalar.dma_start(v_sb[:], v_d[:])
            nc.scalar.dma_start(w1_sb[:], w1_d[:])
            nc.scalar.dma_start(b1_sb[:], b1_d[:])
            nc.scalar.dma_start(one_sb[:], one_d[:])
            nc.scalar.dma_start(b2_sb[:], b2_d[:])
            nc.scalar.dma_start(vta_sb[:], vta_d[:])
            nc.vector.memset(wtile[:], 0.0)
            # pcab pads early (DVE is idle): the shifted-plane DMAs read them
            nc.vector.memset(pcab[0:64, 0, :], 0.0)
            nc.vector.memset(pcab[0:64, H + 1:PH6 + 1, :], 0.0)  # rows 65..67
            nc.vector.memset(pcab[0:64, 1:H + 1, 0:2], 0.0)
            nc.vector.memset(fab[64:128, PLANE_A - 1:PLANE_A], 0.0)
            psw = ps_tile("ps_warm")
            for _ in range(9):
                nc.tensor.matmul(psw[0:64, :], wtile[:, 0:64], wtile[:],
                                 start=True, stop=True)
            nc.scalar.activation(dexp[:], wtile[0:1, 0:1],
                                 mybir.ActivationFunctionType.Exp)


            # ---- M = W1^T @ v (bf16x2) ----
            for cc in range(2):
                psm = ps_tile("ps_m")
                w1s = w1_sb[:, :, cc * 128:(cc + 1) * 128]
                nc.tensor.matmul(psm[:, 0:L], w1s[:, 0, :], v_sb[:, 0, :], start=True, stop=False)
                nc.tensor.matmul(psm[:, 0:L], w1s[:, 0, :], v_sb[:, 1, :], start=False, stop=False)
                nc.tensor.matmul(psm[:, 0:L], w1s[:, 1, :], v_sb[:, 0, :], start=False, stop=True)
                nc.vector.tensor_copy(m_sb[:, cc, 0, :], psm[:, 0:L])
                nc.vector.tensor_tensor(
                    m_sb[:, cc, 1, :], psm[:, 0:L], m_sb[:, cc, 0, :],
                    mybir.AluOpType.subtract,
                )

            # ---- softmax bias column: bias[l] = sum_k b1[k] v[k, l] ----
            for lt in range(2):
                psb = ps_tile("ps_bias")
                vs = v_sb[:, :, lt * 128:(lt + 1) * 128]
                nc.tensor.matmul(psb[:, 0:1], vs[:, 0, :], b1_sb[:], start=True, stop=False)
                nc.tensor.matmul(psb[:, 0:1], vs[:, 1, :], b1_sb[:], start=False, stop=True)
                nc.vector.tensor_copy(bcol_sb[:, lt, :], psb[:, 0:1])

            # keep the PE (and HAM) busy while the first img chunk lands
            for _ in range(6):
                nc.tensor.matmul(psw[0:64, :], wtile[:, 0:64], wtile[:],
                                 start=True, stop=True)

            # ---- attention: scores/exp pipelined one chunk ahead of value ----
            expT = {}
            imtiles = {}

            def finish(i):
                psv = ps_tile("ps_v")
                for lt in range(2):
                    nc.tensor.matmul(
                        psv[0:CK + 1, :], vta_sb[:, lt, :], expT[(i, lt)][:],
                        start=(lt == 0), stop=(lt == 1),
                    )
                nc.vector.tensor_copy(vtmpb[:, i, :], psv[0:CK, :])
                if _CACHE.get("use_plain_recip"):
                    with nc.allow_low_precision(reason="1/denom via bf16"):
                        nc.vector.reciprocal(rdenb[:, i, :], psv[CK:CK + 1, :])
                else:
                    # approx_fast mis-reads large f32 straight from PSUM
                    # (bitwise seed path); stage den to SBUF first.  The two
                    # copies ride the half-idle ACT queue to keep the DVE
                    # chain under the PE's per-chunk pace.
                    nc.scalar.copy(denf[:, i, :], psv[CK:CK + 1, :])
                    nc.vector.reciprocal_approx_fast(rdenf[:, i, :], denf[:, i, :])
                    nc.scalar.copy(rdenb[:, i, :], rdenf[:, i, :])

            def normalize(i):
                psr = ps_tile("ps_r")
                nc.tensor.matmul(psr[0:CK, :], one_sb[:], rdenb[:, i, :],
                                 start=True, stop=True)
                nc.vector.tensor_tensor(
                    pcab[0:CK, 1 + i * 8: 9 + i * 8, 2:PS6],
                    vtmpb[:, i, :], psr[0:CK, :],
                    mybir.AluOpType.mult,
                )

            for j in range(8):
                hw = slice(j * 512, (j + 1) * 512)
                imt = imp.tile([128, 2, 2, 512], BF16, tag="imgc", name="imt")
                imtiles[j] = imt
                for cc in range(2):
                    rows = slice(cc * 128, (cc + 1) * 128)
                    nc.sync.dma_start(imt[:, cc, 0, :], imgh_d[rows, hw])
                    nc.gpsimd.dma_start(imt[:, cc, 1, :], imgl_d[rows, hw])
                for lt in range(2):
                    pst = ps_tile("ps_t")
                    k = 0
                    for cc in range(2):
                        ms = m_sb[:, cc, :, lt * 128:(lt + 1) * 128]
                        for (mh, ih) in ((0, 0), (0, 1), (1, 0)):
                            nc.tensor.matmul(
                                pst[:], ms[:, mh, :], imt[:, cc, ih, :],
                                start=(k == 0), stop=(k == 5),
                            )
                            k += 1
                    et = sm.tile([128, 512], BF16, tag=f"expT{lt}", name=f"expT{lt}")
                    nc.scalar.activation(
                        et[:], pst[:], mybir.ActivationFunctionType.Exp,
                        bias=bcol_sb[:, lt, :],
                    )
                    expT[(j, lt)] = et
                if j == 1:
                    # paced: fires on the ACT queue after chunk 1's exps,
                    # long before the conv needs the weights
                    nc.scalar.dma_start(w2sb[:], w2_d[:])
                if j == 3:
                    # img plane pads: after the early img-lo triggers (so they
                    # don't delay chunk DMAs) but well before the conv reads
                    for p in (pc0, pc1):
                        nc.gpsimd.memset(p[:, 0, :], 0.0)
                        nc.gpsimd.memset(p[:, H + 1, :], 0.0)
                        nc.gpsimd.memset(p[:, H + 2, :], 0.0)
                        nc.gpsimd.memset(p[:, 1:H + 1, 0:2], 0.0)
                if j > 0:
                    finish(j - 1)
                if j > 2:
                    normalize(j - 3)
                # fill conv img planes for this chunk (rows 8j+1 .. 8j+8);
                # emitted after the normalize chain so the DVE prioritizes it
                for cc in range(2):
                    nc.vector.tensor_copy(
                        [pc0, pc1][cc][:, 1 + j * 8: 9 + j * 8, 2:PS6],
                        imt[:, cc, 0, :],
                    )
            finish(7)


            # ---- 3x3 conv schedule ----
            pf0 = pc0[:].rearrange("p a b -> p (a b)")
            pf1 = pc1[:].rearrange("p a b -> p (a b)")
            wsched = []
            for t in range(9):
                for c in range(2):
                    wsched.append((128, [pf0, pf1][c], t // 3, t % 3))
            for t0 in VPAIR_AB:
                wsched.append((128, fab, t0 // 3, t0 % 3))
            wsched.append((128, fcd, 0, 2))   # pair (2, 5)
            wsched.append((128, fab, 2, 2))   # tap 8 (weight rows 64..127 zero)
            assert len(wsched) == NWCH

            def conv_pair(ot, bp, pscs, w_lo, w_hi, drain):
                ocols = slice(ot * 128, (ot + 1) * 128)
                blks = BLOCKS[2 * bp: 2 * bp + 2]
                for w in range(w_lo, w_hi):
                    kk, src, dy, dx = wsched[w]
                    lhsT = w2sb[0:kk, w, ocols]
                    for bi, (y0, r) in enumerate(blks):
                        n = (r - 1) * PS6 + W
                        base = (y0 + dy) * PS6 + dx + 1
                        nc.tensor.matmul(
                            pscs[bi][:, 0:n], lhsT, src[0:kk, base:base + n],
                            start=(w == 0), stop=(w == NWCH - 1),
                        )
                if drain:
                    last = (ot == 1 and bp == 4)
                    for bi, (y0, r) in enumerate(blks):
                        outt = outp.tile([128, r, W], BF16, tag="outt", name="outt")
                        srcv = pscs[bi].rearrange("p (a b) -> p a b", b=PS6)[:, 0:r, 0:W]
                        if last and bi == 1:
                            # final tile: DVE drain + sync-queue DMA run in
                            # parallel with the ACT drain of its sibling
                            nc.vector.tensor_scalar_add(
                                outt[:], srcv, b2_sb[:, ot, :])
                            nc.sync.dma_start(
                                out_d[ocols, y0 * W:(y0 + r) * W], outt[:])
                        else:
                            nc.scalar.activation(
                                outt[:], srcv,
                                mybir.ActivationFunctionType.Identity,
                                bias=b2_sb[:, ot, :],
                            )
                            (nc.sync if last else nc.gpsimd).dma_start(
                                out_d[ocols, y0 * W:(y0 + r) * W], outt[:],
                            )

            def conv_pscs(ot, bp):
                return [ps_pool.tile([128, 7 * PS6], F32, tag="ps",
                                     name=f"psc{ot}_{bp}_{bi}", uniquify=True)
                        for bi in range(2)]

            # first block-pair's img matmuls cover the tail normalize latency;
            # all psr matmuls MUST precede any conv value-tap matmul (the
            # value taps wait on mult(7) -> psr(7): emitting psr later would
            # deadlock the PE FIFO)
            pscs00 = conv_pscs(0, 0)
            conv_pair(0, 0, pscs00, 0, 18, drain=False)
            for j in (5, 6, 7):
                normalize(j)

            # ---- shifted value-plane copies (partition halves via DMA) ----
            # split into block-pair-aligned row ranges: subtile deps let each
            # piece fire as soon as its source rows are normalized, so the
            # (serialized) SBUF->SBUF DMA streams during the attention loop
            cuts = [0, 1056, 1980, 2904, 3828, PLANE_A]
            for a, b in zip(cuts[:-1], cuts[1:]):
                nc.sync.dma_start(fab[64:128, a:min(b, PLANE_A - 1)],
                                  fab[0:64, a + 1:min(b + 1, PLANE_A)])
                bc = min(b, PLANE)
                if a < bc:
                    nc.scalar.dma_start(fcd[0:64, a:bc], fab[0:64, a:bc])
                bh = min(b, PLANE_A - PS6)
                if a < bh:
                    nc.gpsimd.dma_start(fcd[64:128, a:bh],
                                        fab[0:64, a + PS6:bh + PS6])

            pscs01 = conv_pscs(0, 1)
            conv_pair(0, 1, pscs01, 0, 18, drain=False)
            conv_pair(0, 0, pscs00, 18, NWCH, drain=True)
            conv_pair(0, 1, pscs01, 18, NWCH, drain=True)
            for bp in range(2, 5):
                conv_pair(0, bp, conv_pscs(0, bp), 0, NWCH, drain=True)
            for bp in range(5):
                conv_pair(1, bp, conv_pscs(1, bp), 0, NWCH, drain=True)

    nc.compile()
    return nc


def _prep_in_maps_v5(img_embedding, v_embedding, W1, b1, W2, b2):
    # host-side layout prep (no math beyond dtype cast / transpose / pack)
    w2t = np.ascontiguousarray(
        W2.transpose(2, 3, 1, 0).reshape(9, CIN + CK, COUT).astype(np.float32)
    )
    w2p = np.zeros((128, NWCH, COUT), BF)
    for t in range(9):
        w2p[:, 2 * t + 0, :] = w2t[t, 0:128, :].astype(BF)
        w2p[:, 2 * t + 1, :] = w2t[t, 128:256, :].astype(BF)
    for i, t0 in enumerate(VPAIR_AB):
        w2p[0:64, 18 + i, :] = w2t[t0, 256:320, :].astype(BF)
        w2p[64:128, 18 + i, :] = w2t[t0 + 1, 256:320, :].astype(BF)
    w2p[0:64, 21, :] = w2t[2, 256:320, :].astype(BF)
    w2p[64:128, 21, :] = w2t[5, 256:320, :].astype(BF)
    w2p[0:64, 22, :] = w2t[8, 256:320, :].astype(BF)

    w1h, w1l = _split_bf16x2(np.asarray(W1, np.float32))
    w12 = np.stack([w1h, w1l], axis=1)          # [64, 2, 256]
    b1f = np.asarray(b1, np.float32).reshape(CK, 1).astype(BF)
    one64 = np.ones((1, CK), BF)
    b2f = np.ascontiguousarray(
        np.asarray(b2, np.float32).reshape(2, 128).transpose(1, 0).reshape(128, 2, 1)
    )

    in_maps = []
    for bb in range(B):
        img = np.asarray(img_embedding[bb], np.float32).reshape(CIN, HW)
        ih, il = _split_bf16x2(img)
        v32 = np.asarray(v_embedding[bb], np.float32)
        vh, vl = _split_bf16x2(v32)
        v2p = np.stack([vh, vl], axis=1)        # [64, 2, 256]
        vta = np.ones((L, CK + 1), BF)
        vta[:, 0:CK] = v32.T.astype(BF)
        vtap = np.ascontiguousarray(
            vta.reshape(2, 128, CK + 1).transpose(1, 0, 2)
        )                                        # [128, 2, 65]
        in_maps.append(
            {
                "img_hi": np.ascontiguousarray(ih),
                "img_lo": np.ascontiguousarray(il),
                "v2p": np.ascontiguousarray(v2p),
                "vtap": vtap,
                "w12p": np.ascontiguousarray(w12),
                "b1": b1f,
                "one64": one64,
                "w2p23": np.ascontiguousarray(w2p),
                "b2p": b2f,
            }
        )
    return in_maps


def _build_nc_v4():
    nc = bacc.Bacc("TRN2", target_bir_lowering=False, debug=False)

    imgh_d = nc.dram_tensor("img_hi", (CIN, HW), BF16, kind="ExternalInput")
    imgl_d = nc.dram_tensor("img_lo", (CIN, HW), BF16, kind="ExternalInput")
    v_d = nc.dram_tensor("v2", (2, CK, L), BF16, kind="ExternalInput")     # hi, lo
    vta_d = nc.dram_tensor("vta", (L, CK + 1), BF16, kind="ExternalInput")  # v^T | 1
    w1_d = nc.dram_tensor("w12", (2, CK, CIN), BF16, kind="ExternalInput")  # hi, lo
    b1_d = nc.dram_tensor("b1", (CK, 1), BF16, kind="ExternalInput")
    one_d = nc.dram_tensor("one64", (1, CK), BF16, kind="ExternalInput")
    w2_d = nc.dram_tensor("w2p", (128, 27, COUT), BF16, kind="ExternalInput")
    b2_d = nc.dram_tensor("b2", (COUT, 1), F32, kind="ExternalInput")
    out_d = nc.dram_tensor("out", (COUT, HW), F32, kind="ExternalOutput")

    with tile.TileContext(nc) as tc:
        with (
            tc.tile_pool(name="singles", bufs=1) as singles,
            tc.tile_pool(name="sm", bufs=4) as sm,
            tc.tile_pool(name="outp", bufs=4) as outp,
            tc.tile_pool(name="ps_t", bufs=2, space="PSUM") as ps_t,
            tc.tile_pool(name="ps_v", bufs=3, space="PSUM") as ps_v,
            tc.tile_pool(name="ps_c", bufs=2, space="PSUM") as ps_c,
        ):
            # ---- resident tensors ----
            pc0 = singles.tile([128, PH, PS], BF16)
            pc1 = singles.tile([128, PH, PS], BF16)
            pc2 = singles.tile([CK, PH, PS], BF16)
            pci = [pc0, pc1]
            imgc = singles.tile([128, 2, 2, HW], BF16)  # [cc, hi/lo, hw]
            w2sb = singles.tile([128, 27, COUT], BF16)
            vta_sb = singles.tile([128, 2, CK + 1], BF16)
            v_sb = singles.tile([CK, 2, L], BF16)
            w1_sb = singles.tile([CK, 2, CIN], BF16)
            b1_sb = singles.tile([CK, 1], BF16)
            one_sb = singles.tile([1, CK], BF16)
            b2_sb = singles.tile([128, 2, 1], F32)
            m_sb = singles.tile([128, 2, 2, L], BF16)   # [cc, hi/lo, l]
            bcol_sb = singles.tile([128, 2, 1], F32)    # softmax bias, per l-tile

            # ---- small input DMAs on the scalar queue (scores path first) ----
            nc.scalar.dma_start(v_sb[:], v_d.rearrange("h k l -> k h l"))
            nc.scalar.dma_start(w1_sb[:], w1_d.rearrange("h k c -> k h c"))
            nc.scalar.dma_start(b1_sb[:], b1_d[:])
            nc.scalar.dma_start(one_sb[:], one_d[:])
            nc.scalar.dma_start(b2_sb[:], b2_d.rearrange("(t p) x -> p t x", p=128))
            nc.scalar.dma_start(vta_sb[:], vta_d.rearrange("(lc p) c -> p lc c", p=128))
            for cc in range(2):
                nc.scalar.dma_start(imgc[:, cc, 0, :], imgh_d[cc * 128:(cc + 1) * 128, :])
                nc.scalar.dma_start(imgc[:, cc, 1, :], imgl_d[cc * 128:(cc + 1) * 128, :])

            # ---- bulk input DMAs on the sync queue ----
            for p in (pc0, pc1, pc2):
                nc.vector.memset(p[:, 0, :], 0.0)        # top pad row
                nc.vector.memset(p[:, H + 1, :], 0.0)    # bottom pad row
                nc.vector.memset(p[:, H + 2, :], 0.0)    # overrun row
                nc.vector.memset(p[:, 1:H + 1, 0:1], 0.0)  # left pad col (= right pad)
            for cc in range(2):
                nc.sync.dma_start(
                    pci[cc][:, 1:H + 1, 1:PS],
                    imgh_d[cc * 128:(cc + 1) * 128, :].rearrange("p (h w) -> p h w", w=W),
                )
            nc.sync.dma_start(w2sb[:], w2_d[:])

            # ---- M = W1^T @ v (bf16x2), split into hi/lo planes ----
            for cc in range(2):
                ps = ps_t.tile([128, 512], F32, tag="pst", name="ps_m")
                w1s = w1_sb[:, :, cc * 128:(cc + 1) * 128]
                nc.tensor.matmul(ps[:, 0:L], w1s[:, 0, :], v_sb[:, 0, :], start=True, stop=False)
                nc.tensor.matmul(ps[:, 0:L], w1s[:, 0, :], v_sb[:, 1, :], start=False, stop=False)
                nc.tensor.matmul(ps[:, 0:L], w1s[:, 1, :], v_sb[:, 0, :], start=False, stop=True)
                nc.vector.tensor_copy(m_sb[:, cc, 0, :], ps[:, 0:L])
                nc.vector.tensor_tensor(
                    m_sb[:, cc, 1, :], ps[:, 0:L], m_sb[:, cc, 0, :],
                    mybir.AluOpType.subtract,
                )

            # ---- softmax bias column: bias[l] = sum_k b1[k] v[k, l] ----
            for lt in range(2):
                psc0 = ps_v.tile([128, 512], F32, tag="psv", name="ps_bias")
                vs = v_sb[:, :, lt * 128:(lt + 1) * 128]
                nc.tensor.matmul(psc0[:, 0:1], vs[:, 0, :], b1_sb[:], start=True, stop=False)
                nc.tensor.matmul(psc0[:, 0:1], vs[:, 1, :], b1_sb[:], start=False, stop=True)
                nc.vector.tensor_copy(bcol_sb[:, lt, :], psc0[:, 0:1])

            # ---- attention, per 512-pixel chunk ----
            for j in range(8):
                hw = slice(j * 512, (j + 1) * 512)
                expT = []
                for lt in range(2):
                    pst = ps_t.tile([128, 512], F32, tag="pst")
                    k = 0
                    for cc in range(2):
                        ms = m_sb[:, cc, :, lt * 128:(lt + 1) * 128]
                        for (mh, ih) in ((0, 0), (0, 1), (1, 0)):
                            nc.tensor.matmul(
                                pst[:], ms[:, mh, :], imgc[:, cc, ih, hw],
                                start=(k == 0), stop=(k == 5),
                            )
                            k += 1
                    # exp(scores + b1@v) with the bias fused as per-partition ACT bias
                    et = sm.tile([128, 512], BF16, tag=f"expT{lt}", name=f"expT{lt}")
                    nc.scalar.activation(
                        et[:], pst[:], mybir.ActivationFunctionType.Exp,
                        bias=bcol_sb[:, lt, :],
                    )
                    expT.append(et)
                # value (rows 0:64) + softmax denominator (row 64)
                psv = ps_v.tile([CK + 1, 512], F32, tag="psv", name="psv")
                for lt in range(2):
                    nc.tensor.matmul(
                        psv[:], vta_sb[:, lt, :], expT[lt][:],
                        start=(lt == 0), stop=(lt == 1),
                    )
                rden = sm.tile([1, 512], BF16, tag="rden")
                with nc.allow_low_precision(reason="1/denom broadcast via bf16 matmul"):
                    nc.vector.reciprocal(rden[:], psv[CK:CK + 1, :])
                vtmp = sm.tile([CK, 512], F32, tag="vtmp")
                nc.vector.tensor_copy(vtmp[:], psv[0:CK, :])
                # broadcast 1/den across the 64 value partitions via K=1 matmul
                psr = ps_v.tile([CK, 512], F32, tag="psv", name="psr")
                nc.tensor.matmul(psr[:], one_sb[:], rden[:], start=True, stop=True)
                nc.vector.tensor_tensor(
                    pc2[:, 1 + j * 8: 9 + j * 8, 1:PS], vtmp[:], psr[:],
                    mybir.AluOpType.mult,
                )

            # ---- 3x3 conv: 9 shifted matmuls x 3 channel chunks ----
            pf = [p[:].rearrange("p a b -> p (a b)") for p in (pc0, pc1, pc2)]
            for ot in range(2):
                for y0, r in BLOCKS:
                    n = (r - 1) * PS + W  # contiguous window length
                    psc = ps_c.tile([128, 7 * PS], F32)
                    k = 0
                    for tap in range(9):
                        dy, dx = tap // 3, tap % 3
                        base = (y0 + dy) * PS + dx
                        for c in range(3):
                            kk = 128 if c < 2 else CK
                            lhsT = w2sb[0:kk, tap * 3 + c, ot * 128:(ot + 1) * 128]
                            nc.tensor.matmul(
                                psc[:, 0:n], lhsT, pf[c][0:kk, base:base + n],
                                start=(k == 0), stop=(k == 26),
                            )
                            k += 1
                    outt = outp.tile([128, r, W], F32, tag="outt")
                    src = psc.rearrange("p (a b) -> p a b", b=PS)[:, 0:r, 0:W]
                    nc.scalar.activation(
                        outt[:], src, mybir.ActivationFunctionType.Identity,
                        bias=b2_sb[:, ot, :],
                    )
                    nc.sync.dma_start(
                        out_d[ot * 128:(ot + 1) * 128, y0 * W:(y0 + r) * W],
                        outt[:],
                    )

    nc.compile()
    return nc


def _prep_in_maps_v4(img_embedding, v_embedding, W1, b1, W2, b2):
    # host-side layout prep (no math beyond dtype cast / transpose / pack)
    w2t = np.ascontiguousarray(
        W2.transpose(2, 3, 1, 0).reshape(9, CIN + CK, COUT).astype(np.float32)
    )
    w2p = np.zeros((128, 27, COUT), BF)
    for t in range(9):
        w2p[:, t * 3 + 0, :] = w2t[t, 0:128, :].astype(BF)
        w2p[:, t * 3 + 1, :] = w2t[t, 128:256, :].astype(BF)
        w2p[0:CK, t * 3 + 2, :] = w2t[t, 256:320, :].astype(BF)
    w1h, w1l = _split_bf16x2(np.asarray(W1, np.float32))
    w12 = np.stack([w1h, w1l])
    b1f = np.asarray(b1, np.float32).reshape(CK, 1).astype(BF)
    one64 = np.ones((1, CK), BF)
    b2f = np.ascontiguousarray(np.asarray(b2, np.float32).reshape(COUT, 1))

    in_maps = []
    for bb in range(B):
        img = np.asarray(img_embedding[bb], np.float32).reshape(CIN, HW)
        ih, il = _split_bf16x2(img)
        v32 = np.asarray(v_embedding[bb], np.float32)
        vh, vl = _split_bf16x2(v32)
        vta = np.ones((L, CK + 1), BF)
        vta[:, 0:CK] = v32.T.astype(BF)
        in_maps.append(
            {
                "img_hi": np.ascontiguousarray(ih),
                "img_lo": np.ascontiguousarray(il),
                "v2": np.stack([vh, vl]),
                "vta": vta,
                "w12": w12,
                "b1": b1f,
                "one64": one64,
                "w2p": w2p,
                "b2": b2f,
            }
        )
    return in_maps


def _split_bf16x2(a):
    hi = a.astype(BF)
    lo = (a - hi.astype(np.float32)).astype(BF)
    return hi, lo


def _round_f32r(a):
    """Round-to-nearest-even fp32 -> fp32r (11-bit mantissa, low 12 bits zero)."""
    u = np.ascontiguousarray(a, dtype=np.float32).view(np.uint32)
    u = (u + 0x7FF + ((u >> 12) & 1)) & np.uint32(0xFFFFF000)
    return u.view(np.float32)


def _build_nc_v3():
    nc = bacc.Bacc("TRN2", target_bir_lowering=False, debug=False)

    img_d = nc.dram_tensor("img", (CIN, HW), F32R, kind="ExternalInput")
    v_d = nc.dram_tensor("v", (CK, L), F32R, kind="ExternalInput")
    vt_d = nc.dram_tensor("vt_bf", (L, CK), BF16, kind="ExternalInput")
    w1_d = nc.dram_tensor("w1", (CK, CIN), F32R, kind="ExternalInput")
    b1_d = nc.dram_tensor("b1p", (CK, 128), F32R, kind="ExternalInput")
    w2_d = nc.dram_tensor("w2p", (128, 18, COUT), F32R, kind="ExternalInput")
    w2v_d = nc.dram_tensor("w2v", (CK, 9, COUT), BF16, kind="ExternalInput")
    b2_d = nc.dram_tensor("b2", (COUT, 1), F32, kind="ExternalInput")
    zz_d = nc.dram_tensor("zz", (128, PS), F32R, kind="ExternalInput")
    out_d = nc.dram_tensor("out", (COUT, HW), F32, kind="ExternalOutput")

    with tile.TileContext(nc) as tc:
        with (
            tc.tile_pool(name="singles", bufs=1) as singles,
            tc.tile_pool(name="sm", bufs=4) as sm,
            tc.tile_pool(name="outp", bufs=4) as outp,
            tc.tile_pool(name="ps_s", bufs=2, space="PSUM") as ps_s,
            tc.tile_pool(name="ps_v", bufs=2, space="PSUM") as ps_v,
            tc.tile_pool(name="ps_c", bufs=2, space="PSUM") as ps_c,
        ):
            # ---- resident tensors ----
            pc0 = singles.tile([128, PH, PS], F32R)
            pc1 = singles.tile([128, PH, PS], F32R)
            pc2 = singles.tile([CK, PH, PS], BF16)
            pci = [pc0, pc1]
            imgc = singles.tile([128, 2, HW], F32R)  # contiguous img, scores lhsT
            w2sb = singles.tile([128, 18, COUT], F32R)
            w2v_sb = singles.tile([CK, 9, COUT], BF16)
            vt_sb = singles.tile([128, 2, CK], BF16)
            v_sb = singles.tile([CK, L], F32R)
            w1_sb = singles.tile([CK, CIN], F32R)
            b1_sb = singles.tile([CK, 128], F32R)
            b2_sb = singles.tile([128, 2, 1], F32)
            m_sb = singles.tile([128, 2, L], F32R)
            bias_bc = singles.tile([128, L], F32)
            attT = [
                singles.tile([128, HW], BF16, tag=f"attT{lc}", name=f"attT{lc}")
                for lc in range(2)
            ]

            # ---- input DMAs + pad-zeroing ----
            # (DVE memset on float32r is an invalid ISA encoding -- zero the
            # f32r plane pads by DMA from a zeros DRAM tensor instead)
            for p in (pc0, pc1):
                nc.sync.dma_start(p[:, 0, :], zz_d[:])         # top pad row
                nc.sync.dma_start(p[:, H + 1, :], zz_d[:])     # bottom pad row
                nc.sync.dma_start(p[:, H + 2, :], zz_d[:])     # overrun row
                nc.sync.dma_start(p[:, 1:H + 1, 0:1], zz_d[:, 0:H].rearrange("p (w o) -> p w o", o=1))
            nc.vector.memset(pc2[:, 0, :], 0.0)
            nc.vector.memset(pc2[:, H + 1, :], 0.0)
            nc.vector.memset(pc2[:, H + 2, :], 0.0)
            nc.vector.memset(pc2[:, 1:H + 1, 0:1], 0.0)
            for c in range(2):
                nc.sync.dma_start(
                    pci[c][:, 1:H + 1, 1:PS],
                    img_d[c * 128:(c + 1) * 128, :].rearrange("p (h w) -> p h w", w=W),
                )
                nc.sync.dma_start(imgc[:, c, :], img_d[c * 128:(c + 1) * 128, :])

            nc.sync.dma_start(w2sb[:], w2_d[:])
            nc.sync.dma_start(w2v_sb[:], w2v_d[:])
            nc.sync.dma_start(vt_sb[:], vt_d.rearrange("(lc p) c -> p lc c", p=128))
            nc.sync.dma_start(v_sb[:], v_d[:])
            nc.sync.dma_start(w1_sb[:], w1_d[:])
            nc.sync.dma_start(b1_sb[:], b1_d[:])
            nc.sync.dma_start(b2_sb[:], b2_d.rearrange("(t p) x -> p t x", p=128))

            # ---- M = W1^T @ v  [Cin, L], bias broadcast [128, L] ----
            for cc in range(2):
                ps = ps_s.tile([128, L], F32, tag="scores", name="ps_m")
                nc.tensor.matmul(
                    ps[:], w1_sb[:, cc * 128:(cc + 1) * 128], v_sb[:],
                    start=True, stop=True,
                )
                nc.vector.tensor_copy(m_sb[:, cc, :], ps[:])
            # b1 is replicated across all 128 lhsT columns host-side, so this
            # matmul directly materializes bias_row broadcast over partitions
            psb = ps_s.tile([128, L], F32, tag="scores", name="psb")
            nc.tensor.matmul(psb[:], b1_sb[:], v_sb[:], start=True, stop=True)
            nc.vector.tensor_copy(bias_bc[:], psb[:])

            # ---- scores + softmax + transpose, per 128-pixel tile ----
            for i in range(HW // 128):
                ps = ps_s.tile([128, L], F32, tag="scores")
                for cc in range(2):
                    nc.tensor.matmul(
                        ps[:], imgc[:, cc, i * 128:(i + 1) * 128], m_sb[:, cc, :],
                        start=(cc == 0), stop=(cc == 1),
                    )
                nc.vector.tensor_add(ps[:], ps[:], bias_bc[:])
                exp_sb = sm.tile([128, L], F32, tag="exp")
                den = sm.tile([128, 1], F32, tag="den")
                nc.scalar.activation(
                    exp_sb[:], ps[:], mybir.ActivationFunctionType.Exp,
                    accum_out=den[:],
                )
                rden = sm.tile([128, 1], F32, tag="rden")
                nc.vector.reciprocal(rden[:], den[:])
                att = sm.tile([128, L], BF16, tag="att")
                nc.vector.tensor_scalar_mul(att[:], exp_sb[:], rden[:])
                for lc in range(2):
                    nc.sync.dma_start(
                        attT[lc][:, i * 128:(i + 1) * 128],
                        att[:, lc * 128:(lc + 1) * 128],
                        transpose=True,
                    )

            # ---- value = v @ att^T, written into padded plane 2 ----
            for j in range(8):
                psv = ps_v.tile([CK, 8, W], F32)
                for lc in range(2):
                    nc.tensor.matmul(
                        psv[:], vt_sb[:, lc, :], attT[lc][:, j * 512:(j + 1) * 512],
                        start=(lc == 0), stop=(lc == 1),
                    )
                nc.vector.tensor_copy(pc2[:, 1 + j * 8: 9 + j * 8, 1:PS], psv[:])

            # ---- 3x3 conv: 9 shifted matmuls x 3 channel chunks ----
            pf = [p[:].rearrange("p a b -> p (a b)") for p in (pc0, pc1, pc2)]
            for ot in range(2):
                for y0, r in BLOCKS:
                    n = (r - 1) * PS + W  # contiguous window length
                    psc = ps_c.tile([128, 7 * PS], F32)
                    k = 0
                    for tap in range(9):
                        dy, dx = tap // 3, tap % 3
                        base = (y0 + dy) * PS + dx
                        for c in range(3):
                            if c < 2:
                                lhsT = w2sb[:, tap * 2 + c, ot * 128:(ot + 1) * 128]
                            else:
                                lhsT = w2v_sb[:, tap, ot * 128:(ot + 1) * 128]
                            nc.tensor.matmul(
                                psc[:, 0:n], lhsT, pf[c][0:(128 if c < 2 else CK), base:base + n],
                                start=(k == 0), stop=(k == 26),
                            )
                            k += 1
                    outt = outp.tile([128, r, W], F32, tag="outt")
                    src = psc.rearrange("p (a b) -> p a b", b=PS)[:, 0:r, 0:W]
                    nc.scalar.activation(
                        outt[:], src, mybir.ActivationFunctionType.Identity,
                        bias=b2_sb[:, ot, :],
                    )
                    nc.sync.dma_start(
                        out_d[ot * 128:(ot + 1) * 128, y0 * W:(y0 + r) * W],
                        outt[:],
                    )

    nc.compile()
    return nc


def _prep_in_maps_v3(img_embedding, v_embedding, W1, b1, W2, b2):
    # host-side layout prep (no math beyond dtype cast / transpose / pack)
    w2t = np.ascontiguousarray(
        W2.transpose(2, 3, 1, 0).reshape(9, CIN + CK, COUT).astype(np.float32)
    )
    w2p = np.zeros((128, 18, COUT), np.float32)
    for t in range(9):
        w2p[:, t * 2 + 0, :] = w2t[t, 0:128, :]
        w2p[:, t * 2 + 1, :] = w2t[t, 128:256, :]
    w2p = _round_f32r(w2p)
    w2v = np.ascontiguousarray(
        w2t[:, 256:320, :].transpose(1, 0, 2).astype(ml_dtypes.bfloat16)
    )
    w1f = _round_f32r(W1)
    b1p = np.repeat(np.asarray(b1, np.float32).reshape(CK, 1), 128, axis=1)
    b1p = _round_f32r(b1p)
    b2f = np.ascontiguousarray(np.asarray(b2, np.float32).reshape(COUT, 1))
    zz = np.zeros((128, PS), np.float32)

    in_maps = []
    for bb in range(B):
        img = _round_f32r(np.asarray(img_embedding[bb], np.float32).reshape(CIN, HW))
        v32 = np.asarray(v_embedding[bb], np.float32)
        v = _round_f32r(v32)
        vt = np.ascontiguousarray(v32.T.astype(ml_dtypes.bfloat16))
        in_maps.append(
            {
                "img": img,
                "v": v,
                "vt_bf": vt,
                "w1": w1f,
                "b1p": b1p,
                "w2p": w2p,
                "w2v": w2v,
                "b2": b2f,
                "zz": zz,
            }
        )
    return in_maps


def _run(build, prep, key, inputs, trace=False, **kw):
    import sys
    print(f"[kernel] path={key}", file=sys.stderr)
    if key not in _CACHE:
        _CACHE[key] = build()
    in_maps = prep(
        inputs["img_embedding"], inputs["v_embedding"],
        inputs["W1"], inputs["b1"], inputs["W2"], inputs["b2"],
    )
    return bass_utils.run_bass_kernel_spmd(
        _CACHE[key], in_maps, core_ids=list(range(NCORES)), trace=trace, **kw
    )


def run_spmd(inputs, trace=False, **kwargs):
    """v5 (PE-dense) with fallback to silicon-verified v4 then v3."""
    if not _CACHE.get("v5_bad"):
        try:
            return _run(_build_nc_v5, _prep_in_maps_v5, "v5", inputs, trace, **kwargs)
        except Exception:
            _CACHE["v5_bad"] = True
    if not _CACHE.get("v4_bad"):
        try:
            return _run(_build_nc_v4, _prep_in_maps_v4, "v4", inputs, trace, **kwargs)
        except Exception:
            _CACHE["v4_bad"] = True
    return _run(_build_nc_v3, _prep_in_maps_v3, "v3", inputs, trace, **kwargs)


def kernel(**inputs):
    res = run_spmd(inputs)
    out = np.stack([res.results[c]["out"] for c in range(NCORES)])
    return out.reshape(B, COUT, H, W).astype(np.float32)

